# revision 8
# baseline (speedup 1.0000x reference)
"""Multi-head attention Trainium2 Bass kernel.

Problem: B=2, S=2048, D=1024, H=16, HS=64.
Sharding: tensor-parallel over heads — each of 8 cores computes 2 heads
(128 contiguous output-feature columns) for both batches; host concatenates.

v4 schedule, built around measured hardware behavior:
  - ACT exp stream: ~1114 ns per [128,1024] PSUM->SBUF tile, 2 per k-step
    (2228 ns cadence) — the attention-phase limiter.
  - PE issues N=512 bf16 matmuls at ~215 ns; per k-step attention needs
    ~1.8 us (4 sim + 4 PV), leaving ~0.4 us slack per step for projection
    work. Total PE work (proj ~47us + attn ~120us) exceeds total ACT work
    (~143us), so the goal is a gap-free PE stream.
  - DMA is packetized per destination row across 16 engines; small rows
    halve throughput. X^T rides [128,1024]/[128,2048] slabs ordered by
    first use; weights are host-packed (bf16, SBUF layout) into one
    [128,3088] tensor so they land in ~2 us on the scalar queue.

Timeline: warmup matmuls ramp the PE p-state and a dummy exp preloads the
ACT table during the DMA lead-in; K/Q projections for (b0, qh0) run with
their contraction matmuls interleaved so each fires as its X^T chunk
arrives; attention starts ~16 us in. All remaining projection tiles are
injected into the attention k-loops in pairs (two psA-pool tiles per
injection keeps the sim double-buffer rotation parity stable), placed so
each lands after its DMA dependency and before its consumer phase. Q/K
bias adds are fused into PSUM->SBUF copies (scalar engine in the prologue,
DVE during attention); split projection tiles merge halves with one
scalar_tensor_tensor. V' keeps the [V_h0|1|V_h1|1] ones-column layout so
PV accumulation yields the softmax denominator as row 65; the host
divides and transposes during assembly.
"""

import sys

sys.path.insert(0, "/opt/trn_rl_repo")

import ml_dtypes
import numpy as np

import concourse.bass as bass
import concourse.mybir as mybir
import concourse.tile as tile
from concourse import bacc
from concourse import bass_utils

B, S, D = 2, 2048, 1024
H, HS = 16, 64
NCORES = 8
NTOK = B * S                  # 4096
FPC = (H // NCORES) * HS      # 128 output-feature cols per core (2 heads)
TT = 512                      # token tile for projections
NTT = NTOK // TT              # 8
NCH = D // 128                # 8 contraction chunks
QT = 512                      # q tile (one matmul / psum bank)
QH = 2 * QT                   # 1024-wide q half
KT = 128                      # k chunk in attention
NKT = S // KT                 # 16
VW = 2 * (HS + 1)             # 130: [V_h0 | 1 | V_h1 | 1] columns
WALLW = 2 * NCH * FPC + NCH * VW   # 3088 packed weight cols

F32 = mybir.dt.float32
BF16 = mybir.dt.bfloat16

_NC_CACHE = {}


def build_nc():
    nc = bacc.Bacc("TRN2", target_bir_lowering=False, debug=False, num_devices=NCORES)
    xt = nc.dram_tensor("xt", [D, NTOK], BF16, kind="ExternalInput").ap()
    wall = nc.dram_tensor("wall", [128, WALLW], BF16, kind="ExternalInput").ap()
    bcol = nc.dram_tensor("bcol", [FPC, 2], F32, kind="ExternalInput").ap()
    brow = nc.dram_tensor("brow", [1, VW + KT], BF16, kind="ExternalInput").ap()
    out = nc.dram_tensor("out", [2 * (HS + 1), NTOK], F32, kind="ExternalOutput").ap()

    with tile.TileContext(nc) as tc:
        with (
            tc.tile_pool(name="persist", bufs=1) as pp,
            tc.tile_pool(name="work", bufs=2) as wk_pool,
            tc.tile_pool(name="psA", bufs=2, space="PSUM") as psA,
            tc.tile_pool(name="psB", bufs=2, space="PSUM") as psB,
        ):
            wall_sb = pp.tile([128, WALLW], BF16)
            bcol_sb = pp.tile([128, 2], F32)
            brow_sb = pp.tile([1, VW + KT], BF16)
            warm_sb = pp.tile([128, TT], BF16)
            junk_sb = pp.tile([128, TT], BF16)
            xtc = [pp.tile([128, NTOK], BF16, name=f"xt_{c}") for c in range(NCH)]
            qt_sb = pp.tile([128, NTOK], BF16)   # Q^T: [feat(2 heads), tok]
            kt_sb = pp.tile([128, NTOK], BF16)   # K^T
            vp_sb = pp.tile([128, (NTOK // 128) * VW], BF16)  # V' [tok128,130] chunks

            def wk_c(c):
                return wall_sb[:, c * FPC : (c + 1) * FPC]

            def wq_c(c):
                return wall_sb[:, NCH * FPC + c * FPC : NCH * FPC + (c + 1) * FPC]

            def wv_c(c):
                o = 2 * NCH * FPC
                return wall_sb[:, o + c * VW : o + (c + 1) * VW]

            bq_ap = bcol_sb[:, 0:1]
            bk_ap = bcol_sb[:, 1:2]
            bv_b = brow_sb[:, 0:VW]
            ones_b = brow_sb[:, VW:]

            # ---------------- DMAs -------------------------------------------
            nc.scalar.dma_start(wall_sb[:], wall[:, :])
            nc.scalar.dma_start(bcol_sb[:], bcol[:, :])
            nc.scalar.dma_start(brow_sb[:], brow[:, :])
            # X^T need-ordered: b0 as 2KB-row halves, b1 as 4KB-row slabs.
            for c in range(NCH):
                nc.sync.dma_start(xtc[c][:, 0:QH], xt[c * 128 : (c + 1) * 128, 0:QH])
            for c in range(NCH):
                nc.sync.dma_start(xtc[c][:, QH:S], xt[c * 128 : (c + 1) * 128, QH:S])
            for c in range(NCH):
                nc.sync.dma_start(xtc[c][:, S : 2 * S], xt[c * 128 : (c + 1) * 128, S : 2 * S])

            # ---------------- warmup (PE p-state ramp + ACT exp table) -------
            nc.gpsimd.memset(warm_sb[:], 0.0)
            warm_ps = psA.tile([128, TT], F32, name="warm", tag="psA", padded_shape=[128, QH])
            for _ in range(5):
                nc.tensor.matmul(warm_ps[:], warm_sb[:, 0:128], warm_sb[:], start=True, stop=True)
            nc.scalar.activation(junk_sb[:], warm_ps[:], mybir.ActivationFunctionType.Exp)

            # ---------------- projection pieces ------------------------------
            def qk_mms(t, wsel, pool, c_lo=0, c_hi=NCH):
                tsl = slice(t * TT, (t + 1) * TT)
                ps = pool.tile([128, TT], F32, name=f"pj{t}_{c_lo}_{wsel.__name__}",
                               tag=pool.name, padded_shape=[128, QH])
                def mm(c):
                    nc.tensor.matmul(ps[:], wsel(c), xtc[c][:, tsl],
                                     start=(c == c_lo), stop=(c == c_hi - 1))
                return ps, mm

            def emit_v_chunk(ch, pool):
                psv = pool.tile([128, VW], F32, name=f"pv_{ch}", tag=pool.name,
                                padded_shape=[128, QH])
                for c in range(NCH):
                    nc.tensor.matmul(
                        psv[:], xtc[c][:, ch * 128 : (ch + 1) * 128], wv_c(c),
                        start=(c == 0), stop=False,
                    )
                nc.tensor.matmul(psv[:], ones_b, bv_b, start=False, stop=True)
                nc.vector.tensor_copy(vp_sb[:, ch * VW : (ch + 1) * VW], psv[:])

            # ---------------- P0: b0 K t0-3 + Q t0,t1 c-interleaved ----------
            p0 = []
            for t, wsel, pool in ((0, wk_c, psA), (1, wk_c, psA),
                                  (0, wq_c, psB), (1, wq_c, psB)):
                p0.append(qk_mms(t, wsel, pool) + (t, wsel))
            for c in range(NCH):
                for ps, mm, t, wsel in p0:
                    mm(c)
            for i, (ps, mm, t, wsel) in enumerate(p0):
                tsl = slice(t * TT, (t + 1) * TT)
                if wsel is wk_c:
                    nc.scalar.activation(kt_sb[:, tsl], ps[:],
                                         mybir.ActivationFunctionType.Identity, bias=bk_ap)
                else:
                    nc.vector.tensor_scalar_add(qt_sb[:, tsl], ps[:], bq_ap)
            for ch in (0, 1, 2, 3):
                emit_v_chunk(ch, psB)

            # ---------------- injected units (each allocates ONE psA tile) ---
            stages = {}

            def v_unit(ch):
                def f():
                    emit_v_chunk(ch, psA)
                return f

            def qk_half(t, wsel, bias_ap, dst, half):
                def f():
                    c_lo, c_hi = half * 4, half * 4 + 4
                    ps, mm = qk_mms(t, wsel, psA, c_lo, c_hi)
                    for c in range(c_lo, c_hi):
                        mm(c)
                    tsl = slice(t * TT, (t + 1) * TT)
                    if half == 0:
                        stg = wk_pool.tile([128, TT], F32, name=f"stg_{t}_{wsel.__name__}",
                                           tag="stg", bufs=2)
                        stages[(t, wsel.__name__)] = stg
                        nc.vector.tensor_scalar_add(stg[:], ps[:], bias_ap)
                    else:
                        stg = stages.pop((t, wsel.__name__))
                        nc.vector.tensor_tensor(dst[:, tsl], stg[:], ps[:],
                                                mybir.AluOpType.add)
                return f

            def K_(t, half):
                return qk_half(t, wk_c, bk_ap, kt_sb, half)

            def Q_(t, half):
                return qk_half(t, wq_c, bq_ap, qt_sb, half)

            # Mostly single-unit injections: one extra psA tile per k-step
            # rides the exp(h0) buffer window and is fully hidden; pairs only
            # where the slot budget forces them. Deadlines: V'(ch j) before
            # pv(kt=j); K t-tile before sims of its k-range; Q before its
            # (b, qh) phase.
            SCHED = {
                (0, 0): {
                    0: [v_unit(4)], 1: [v_unit(5)], 2: [v_unit(6)], 3: [v_unit(7)],
                    4: [v_unit(8)], 5: [K_(2, 0), K_(2, 1)], 6: [v_unit(9)],
                    7: [v_unit(10)], 8: [v_unit(11)], 9: [K_(3, 0), K_(3, 1)],
                    10: [v_unit(12)], 11: [v_unit(13)], 12: [v_unit(14)],
                    13: [v_unit(15)], 14: [Q_(2, 0), Q_(2, 1)], 15: [Q_(3, 0), Q_(3, 1)],
                },
                (0, 1): {
                    0: [K_(4, 0)], 1: [K_(4, 1)], 2: [K_(5, 0)], 3: [K_(5, 1)],
                    4: [K_(6, 0)], 5: [K_(6, 1)], 6: [K_(7, 0)], 7: [K_(7, 1)],
                    8: [Q_(4, 0)], 9: [Q_(4, 1)], 10: [Q_(5, 0)], 11: [Q_(5, 1)],
                },
                (1, 0): {
                    0: [v_unit(16)], 1: [v_unit(17)], 2: [v_unit(18)], 3: [v_unit(19)],
                    4: [v_unit(20)], 5: [v_unit(21)], 6: [v_unit(22)], 7: [v_unit(23)],
                    8: [v_unit(24)], 9: [v_unit(25)], 10: [v_unit(26)], 11: [v_unit(27)],
                    12: [v_unit(28), Q_(6, 0)], 13: [v_unit(29), Q_(6, 1)],
                    14: [v_unit(30), Q_(7, 0)], 15: [v_unit(31), Q_(7, 1)],
                },
                (1, 1): {},
            }

            # ---------------- attention --------------------------------------
            def attn_phase(b, qh):
                sched = SCHED[(b, qh)]
                pvp = [
                    psB.tile([HS + 1, QH], F32, name=f"pvp_{b}_{qh}_{h}", tag="psB",
                             padded_shape=[128, QH])
                    for h in range(2)
                ]
                pts = {}

                def pvs(kt):
                    ch = (b * S) // 128 + kt
                    for h in range(2):
                        for qq in range(2):
                            nc.tensor.matmul(
                                pvp[h][:, qq * QT : (qq + 1) * QT],
                                vp_sb[:, ch * VW + h * (HS + 1) : ch * VW + (h + 1) * (HS + 1)],
                                pts[kt][h][:, qq * QT : (qq + 1) * QT],
                                start=(kt == 0), stop=(kt == NKT - 1),
                            )
                    del pts[kt]

                for kt in range(NKT):
                    ksl = b * S + kt * KT
                    sims = [
                        psA.tile([128, QH], F32, name=f"sim_{b}_{qh}_{kt}_{h}", tag="psA",
                                 padded_shape=[128, QH])
                        for h in range(2)
                    ]
                    for qq in range(2):
                        for h in range(2):
                            hp = h * HS
                            qsl = b * S + qh * QH + qq * QT
                            nc.tensor.matmul(
                                sims[h][:, qq * QT : (qq + 1) * QT],
                                kt_sb[hp : hp + HS, ksl : ksl + KT],
                                qt_sb[hp : hp + HS, qsl : qsl + QT],
                                start=True, stop=True,
                                tile_position=(hp, 0),
                            )
                    cur = []
                    for h in range(2):
                        pt = wk_pool.tile([128, QH], BF16, name=f"pt_{b}_{qh}_{kt}_{h}",
                                          tag="pt", bufs=6)
                        nc.scalar.activation(pt[:], sims[h][:],
                                             mybir.ActivationFunctionType.Exp,
                                             scale=1.0 / np.sqrt(HS))
                        cur.append(pt)
                    pts[kt] = cur
                    if kt > 0:
                        pvs(kt - 1)
                    for fn in sched.get(kt, ()):
                        fn()
                pvs(NKT - 1)
                for h in range(2):
                    ot = wk_pool.tile([HS + 1, QH], F32, name=f"ot_{b}_{qh}_{h}",
                                      tag="ot", bufs=2)
                    nc.vector.tensor_copy(ot[:], pvp[h][:])
                    nc.sync.dma_start(
                        out[h * (HS + 1) : (h + 1) * (HS + 1), b * S + qh * QH : b * S + (qh + 1) * QH],
                        ot[:],
                    )

            attn_phase(0, 0)
            attn_phase(0, 1)
            attn_phase(1, 0)
            attn_phase(1, 1)

    nc.compile()
    return nc


def get_nc():
    if "nc" not in _NC_CACHE:
        _NC_CACHE["nc"] = build_nc()
    return _NC_CACHE["nc"]


def make_in_maps(seq_input, WQ, bQ, WK, bK, WV, bV):
    x = np.asarray(seq_input, dtype=np.float32).reshape(NTOK, D)
    xt = np.ascontiguousarray(x.T).astype(ml_dtypes.bfloat16)
    in_maps = []
    for c in range(NCORES):
        lo, hi = c * FPC, (c + 1) * FPC
        wall = np.zeros((128, WALLW), dtype=np.float32)
        for ch in range(NCH):
            rs = slice(ch * 128, (ch + 1) * 128)
            wall[:, ch * FPC : (ch + 1) * FPC] = WK[rs, lo:hi]
            wall[:, NCH * FPC + ch * FPC : NCH * FPC + (ch + 1) * FPC] = WQ[rs, lo:hi]
            o = 2 * NCH * FPC
            wall[:, o + ch * VW : o + ch * VW + HS] = WV[rs, lo : lo + HS]
            wall[:, o + ch * VW + HS + 1 : o + ch * VW + 2 * HS + 1] = WV[rs, lo + HS : hi]
        bcol = np.stack([bQ[lo:hi], bK[lo:hi]], axis=1).astype(np.float32)
        brow = np.zeros((1, VW + KT), dtype=np.float32)
        brow[0, 0:HS] = bV[lo : lo + HS]
        brow[0, HS] = 1.0
        brow[0, HS + 1 : 2 * HS + 1] = bV[lo + HS : hi]
        brow[0, 2 * HS + 1] = 1.0
        brow[0, VW:] = 1.0
        in_maps.append(
            {
                "xt": xt,
                "wall": wall.astype(ml_dtypes.bfloat16),
                "bcol": np.ascontiguousarray(bcol),
                "brow": brow.astype(ml_dtypes.bfloat16),
            }
        )
    return in_maps


def run(in_maps, trace=False):
    nc = get_nc()
    return bass_utils.run_bass_kernel_spmd(nc, in_maps, core_ids=list(range(NCORES)), trace=trace)


def kernel(seq_input, WQ, bQ, WK, bK, WV, bV):
    in_maps = make_in_maps(
        np.asarray(seq_input, np.float32),
        np.asarray(WQ, np.float32), np.asarray(bQ, np.float32),
        np.asarray(WK, np.float32), np.asarray(bK, np.float32),
        np.asarray(WV, np.float32), np.asarray(bV, np.float32),
    )
    res = run(in_maps)
    parts = []
    for c in range(NCORES):
        o = res.results[c]["out"]  # [130, 4096] feature-major, unnormalized
        for h in range(2):
            num = o[h * (HS + 1) : h * (HS + 1) + HS, :]      # [64, 4096]
            den = o[h * (HS + 1) + HS, :]                     # [4096]
            parts.append((num / den).T)                       # [4096, 64]
    full = np.concatenate(parts, axis=1)  # [4096, 1024]
    return full.reshape(B, S, H * HS)


# revision 18
# speedup vs baseline: 1.0240x; 1.0240x over previous
"""Multi-head attention Trainium2 Bass kernel.

Problem: B=2, S=2048, D=1024, H=16, HS=64.
Sharding: tensor-parallel over heads — each of 8 cores computes 2 heads
(128 contiguous output-feature columns) for both batches; host concatenates.

v4 schedule, built around measured hardware behavior:
  - ACT exp stream: ~1114 ns per [128,1024] PSUM->SBUF tile, 2 per k-step
    (2228 ns cadence) — the attention-phase limiter.
  - PE issues N=512 bf16 matmuls at ~215 ns; per k-step attention needs
    ~1.8 us (4 sim + 4 PV), leaving ~0.4 us slack per step for projection
    work. Total PE work (proj ~47us + attn ~120us) exceeds total ACT work
    (~143us), so the goal is a gap-free PE stream.
  - DMA is packetized per destination row across 16 engines; small rows
    halve throughput. X^T rides [128,1024]/[128,2048] slabs ordered by
    first use; weights are host-packed (bf16, SBUF layout) into one
    [128,3088] tensor so they land in ~2 us on the scalar queue.

Timeline: warmup matmuls ramp the PE p-state and a dummy exp preloads the
ACT table during the DMA lead-in; K/Q projections for (b0, qh0) run with
their contraction matmuls interleaved so each fires as its X^T chunk
arrives; attention starts ~16 us in. All remaining projection tiles are
injected into the attention k-loops in pairs (two psA-pool tiles per
injection keeps the sim double-buffer rotation parity stable), placed so
each lands after its DMA dependency and before its consumer phase. Q/K
bias adds are fused into PSUM->SBUF copies (scalar engine in the prologue,
DVE during attention); split projection tiles merge halves with one
scalar_tensor_tensor. V' keeps the [V_h0|1|V_h1|1] ones-column layout so
PV accumulation yields the softmax denominator as row 65; the host
divides and transposes during assembly.
"""

import sys

sys.path.insert(0, "/opt/trn_rl_repo")

import ml_dtypes
import numpy as np

import concourse.bass as bass
import concourse.mybir as mybir
import concourse.tile as tile
from concourse import bacc
from concourse import bass_utils

B, S, D = 2, 2048, 1024
H, HS = 16, 64
NCORES = 8
NTOK = B * S                  # 4096
FPC = (H // NCORES) * HS      # 128 output-feature cols per core (2 heads)
TT = 512                      # token tile for projections
NTT = NTOK // TT              # 8
NCH = D // 128                # 8 contraction chunks
QT = 512                      # q tile (one matmul / psum bank)
QH = 2 * QT                   # 1024-wide q half
KT = 128                      # k chunk in attention
NKT = S // KT                 # 16
VW = 2 * (HS + 1)             # 130: [V_h0 | 1 | V_h1 | 1] columns
WALLW = 2 * NCH * FPC + NCH * VW   # 3088 packed weight cols

F32 = mybir.dt.float32
BF16 = mybir.dt.bfloat16

_NC_CACHE = {}


def build_nc():
    nc = bacc.Bacc("TRN2", target_bir_lowering=False, debug=False, num_devices=NCORES)
    # X^T host-packed chunk-major per token range: row p of xt1 holds
    # [chunk0 tokens 0-1023 | chunk1 ... ] so each DMA has 16-32KB rows.
    xt1 = nc.dram_tensor("xt1", [128, NCH * QH], BF16, kind="ExternalInput").ap()
    xt2 = nc.dram_tensor("xt2", [128, NCH * QH], BF16, kind="ExternalInput").ap()
    xt3 = nc.dram_tensor("xt3", [128, NCH * S], BF16, kind="ExternalInput").ap()
    wall = nc.dram_tensor("wall", [128, WALLW], BF16, kind="ExternalInput").ap()
    bcol = nc.dram_tensor("bcol", [FPC, 2], F32, kind="ExternalInput").ap()
    brow = nc.dram_tensor("brow", [1, VW + KT], BF16, kind="ExternalInput").ap()
    out = nc.dram_tensor("out", [2 * (HS + 1), NTOK], F32, kind="ExternalOutput").ap()

    with tile.TileContext(nc) as tc:
        with (
            tc.tile_pool(name="persist", bufs=1) as pp,
            tc.tile_pool(name="work", bufs=2) as wk_pool,
            tc.tile_pool(name="psA", bufs=2, space="PSUM") as psA,
            tc.tile_pool(name="psB", bufs=2, space="PSUM") as psB,
        ):
            wall_sb = pp.tile([128, WALLW], BF16)
            bcol_sb = pp.tile([128, 2], F32)
            brow_sb = pp.tile([1, VW + KT], BF16)
            warm_sb = pp.tile([128, TT], BF16)
            junk_sb = pp.tile([128, TT], BF16)
            xa1 = pp.tile([128, NCH * QH], BF16)
            xa2 = pp.tile([128, NCH * QH], BF16)
            xa3 = pp.tile([128, NCH * S], BF16)
            qt_sb = pp.tile([128, NTOK], BF16)   # Q^T: [feat(2 heads), tok]
            kt_sb = pp.tile([128, NTOK], BF16)   # K^T
            vp_sb = pp.tile([128, (NTOK // 128) * VW], BF16)  # V' [tok128,130] chunks

            def wk_c(c):
                return wall_sb[:, c * FPC : (c + 1) * FPC]

            def wq_c(c):
                return wall_sb[:, NCH * FPC + c * FPC : NCH * FPC + (c + 1) * FPC]

            def wv_c(c):
                o = 2 * NCH * FPC
                return wall_sb[:, o + c * VW : o + (c + 1) * VW]

            bq_ap = bcol_sb[:, 0:1]
            bk_ap = bcol_sb[:, 1:2]
            bv_b = brow_sb[:, 0:VW]
            ones_b = brow_sb[:, VW:]

            def xs(c, tok, w):
                """X^T AP for contraction chunk c, tokens [tok, tok+w)."""
                if tok < QH:
                    return xa1[:, c * QH + tok : c * QH + tok + w]
                if tok < S:
                    return xa2[:, c * QH + (tok - QH) : c * QH + (tok - QH) + w]
                return xa3[:, c * S + (tok - S) : c * S + (tok - S) + w]

            # ---------------- DMAs -------------------------------------------
            nc.scalar.dma_start(wall_sb[:], wall[:, :])
            nc.scalar.dma_start(bcol_sb[:], bcol[:, :])
            nc.scalar.dma_start(brow_sb[:], brow[:, :])
            # X^T need-ordered, packed layout: chunk-pair DMAs (4KB+ rows)
            # so the first projections start as early pairs land.
            for g in range(4):
                nc.sync.dma_start(xa1[:, g * 2 * QH : (g + 1) * 2 * QH],
                                  xt1[:, g * 2 * QH : (g + 1) * 2 * QH])
            for g in range(2):
                nc.sync.dma_start(xa2[:, g * 4 * QH : (g + 1) * 4 * QH],
                                  xt2[:, g * 4 * QH : (g + 1) * 4 * QH])
            for g in range(2):
                nc.sync.dma_start(xa3[:, g * 4 * S : (g + 1) * 4 * S],
                                  xt3[:, g * 4 * S : (g + 1) * 4 * S])

            # ---------------- warmup (PE p-state ramp + ACT exp table) -------
            nc.gpsimd.memset(warm_sb[:], 0.0)
            warm_ps = psA.tile([128, TT], F32, name="warm", tag="psA", padded_shape=[128, QH])
            for _ in range(5):
                nc.tensor.matmul(warm_ps[:], warm_sb[:, 0:128], warm_sb[:], start=True, stop=True)
            nc.scalar.activation(junk_sb[:], warm_ps[:], mybir.ActivationFunctionType.Exp)

            # ---------------- projection pieces ------------------------------
            def qk_mms(t, wsel, pool, c_lo=0, c_hi=NCH):
                tsl = slice(t * TT, (t + 1) * TT)
                ps = pool.tile([128, TT], F32, name=f"pj{t}_{c_lo}_{wsel.__name__}",
                               tag=pool.name, padded_shape=[128, QH])
                def mm(c):
                    nc.tensor.matmul(ps[:], wsel(c), xs(c, t * TT, TT),
                                     start=(c == c_lo), stop=(c == c_hi - 1))
                return ps, mm

            def emit_v_chunk(ch, pool):
                psv = pool.tile([128, VW], F32, name=f"pv_{ch}", tag=pool.name,
                                padded_shape=[128, QH])
                for c in range(NCH):
                    nc.tensor.matmul(
                        psv[:], xs(c, ch * 128, 128), wv_c(c),
                        start=(c == 0), stop=False,
                    )
                nc.tensor.matmul(psv[:], ones_b, bv_b, start=False, stop=True)
                nc.vector.tensor_copy(vp_sb[:, ch * VW : (ch + 1) * VW], psv[:])

            # ---------------- P0: b0 K t0-3 + Q t0,t1 c-interleaved ----------
            p0 = []
            for t, wsel, pool in ((0, wk_c, psA), (1, wk_c, psA),
                                  (0, wq_c, psB), (1, wq_c, psB)):
                p0.append(qk_mms(t, wsel, pool) + (t, wsel))
            for c in range(NCH):
                for ps, mm, t, wsel in p0:
                    mm(c)
            for i, (ps, mm, t, wsel) in enumerate(p0):
                tsl = slice(t * TT, (t + 1) * TT)
                if wsel is wk_c:
                    nc.scalar.activation(kt_sb[:, tsl], ps[:],
                                         mybir.ActivationFunctionType.Identity, bias=bk_ap)
                else:
                    nc.vector.tensor_scalar_add(qt_sb[:, tsl], ps[:], bq_ap)
            for ch in (0, 1, 2, 3):
                emit_v_chunk(ch, psB)

            # ---------------- injected units (each allocates ONE psA tile) ---
            stages = {}

            def v_unit(ch):
                def f():
                    emit_v_chunk(ch, psA)
                return f

            def qk_half(t, wsel, bias_ap, dst, half):
                def f():
                    c_lo, c_hi = half * 4, half * 4 + 4
                    ps, mm = qk_mms(t, wsel, psA, c_lo, c_hi)
                    for c in range(c_lo, c_hi):
                        mm(c)
                    tsl = slice(t * TT, (t + 1) * TT)
                    if half == 0:
                        stg = wk_pool.tile([128, TT], F32, name=f"stg_{t}_{wsel.__name__}",
                                           tag="stg", bufs=2)
                        stages[(t, wsel.__name__)] = stg
                        nc.vector.tensor_scalar_add(stg[:], ps[:], bias_ap)
                    else:
                        stg = stages.pop((t, wsel.__name__))
                        nc.vector.tensor_tensor(dst[:, tsl], stg[:], ps[:],
                                                mybir.AluOpType.add)
                return f

            def K_(t, half):
                return qk_half(t, wk_c, bk_ap, kt_sb, half)

            def Q_(t, half):
                return qk_half(t, wq_c, bq_ap, qt_sb, half)

            # Injections in pairs (two psA tiles per slot keeps the sim
            # double-buffer rotation parity stable). Deadlines: V'(ch j)
            # before pv(kt=j); K t-tile before sims of its k-range; Q before
            # its (b, qh) phase. Slots kt0/kt1 are kept free so the exp
            # cadence locks before the first injection.
            SCHED = {
                (0, 0): {
                    2: [v_unit(4), v_unit(5)], 3: [v_unit(6), v_unit(7)],
                    4: [K_(2, 0), K_(2, 1)], 5: [K_(3, 0), K_(3, 1)],
                    6: [v_unit(8), v_unit(9)], 7: [v_unit(10), v_unit(11)],
                    8: [v_unit(12), v_unit(13)], 9: [v_unit(14), v_unit(15)],
                    10: [Q_(2, 0), Q_(2, 1)], 11: [Q_(3, 0), Q_(3, 1)],
                    14: [K_(4, 0), K_(4, 1)], 15: [K_(5, 0), K_(5, 1)],
                },
                (0, 1): {
                    0: [K_(6, 0), K_(6, 1)], 1: [K_(7, 0), K_(7, 1)],
                    2: [Q_(4, 0), Q_(4, 1)], 3: [Q_(5, 0), Q_(5, 1)],
                },
                (1, 0): {
                    0: [v_unit(16), v_unit(17)], 1: [v_unit(18), v_unit(19)],
                    2: [v_unit(20), v_unit(21)], 3: [v_unit(22), v_unit(23)],
                    4: [v_unit(24), v_unit(25)], 5: [v_unit(26), v_unit(27)],
                    6: [v_unit(28), v_unit(29)], 7: [v_unit(30), v_unit(31)],
                    8: [Q_(6, 0), Q_(6, 1)], 9: [Q_(7, 0), Q_(7, 1)],
                },
                (1, 1): {},
            }

            # ---------------- attention --------------------------------------
            def attn_phase(b, qh):
                sched = SCHED[(b, qh)]
                pvp = [
                    psB.tile([HS + 1, QH], F32, name=f"pvp_{b}_{qh}_{h}", tag="psB",
                             padded_shape=[128, QH])
                    for h in range(2)
                ]
                pts = {}

                def pvs(kt):
                    ch = (b * S) // 128 + kt
                    for h in range(2):
                        for qq in range(2):
                            nc.tensor.matmul(
                                pvp[h][:, qq * QT : (qq + 1) * QT],
                                vp_sb[:, ch * VW + h * (HS + 1) : ch * VW + (h + 1) * (HS + 1)],
                                pts[kt][h][:, qq * QT : (qq + 1) * QT],
                                start=(kt == 0), stop=(kt == NKT - 1),
                            )
                    del pts[kt]

                for kt in range(NKT):
                    ksl = b * S + kt * KT
                    sims = [
                        psA.tile([128, QH], F32, name=f"sim_{b}_{qh}_{kt}_{h}", tag="psA",
                                 padded_shape=[128, QH])
                        for h in range(2)
                    ]
                    for qq in range(2):
                        for h in range(2):
                            hp = h * HS
                            qsl = b * S + qh * QH + qq * QT
                            nc.tensor.matmul(
                                sims[h][:, qq * QT : (qq + 1) * QT],
                                kt_sb[hp : hp + HS, ksl : ksl + KT],
                                qt_sb[hp : hp + HS, qsl : qsl + QT],
                                start=True, stop=True,
                                tile_position=(hp, 0),
                            )
                    cur = []
                    for h in range(2):
                        pt = wk_pool.tile([128, QH], BF16, name=f"pt_{b}_{qh}_{kt}_{h}",
                                          tag="pt", bufs=6)
                        nc.scalar.activation(pt[:], sims[h][:],
                                             mybir.ActivationFunctionType.Exp,
                                             scale=1.0 / np.sqrt(HS))
                        cur.append(pt)
                    pts[kt] = cur
                    if kt > 0:
                        pvs(kt - 1)
                    for fn in sched.get(kt, ()):
                        fn()
                pvs(NKT - 1)
                for h in range(2):
                    ot = wk_pool.tile([HS + 1, QH], F32, name=f"ot_{b}_{qh}_{h}",
                                      tag="ot", bufs=2)
                    nc.vector.tensor_copy(ot[:], pvp[h][:])
                    nc.sync.dma_start(
                        out[h * (HS + 1) : (h + 1) * (HS + 1), b * S + qh * QH : b * S + (qh + 1) * QH],
                        ot[:],
                    )

            attn_phase(0, 0)
            attn_phase(0, 1)
            attn_phase(1, 0)
            attn_phase(1, 1)

    nc.compile()
    return nc


def get_nc():
    if "nc" not in _NC_CACHE:
        _NC_CACHE["nc"] = build_nc()
    return _NC_CACHE["nc"]


def make_in_maps(seq_input, WQ, bQ, WK, bK, WV, bV):
    x = np.asarray(seq_input, dtype=np.float32).reshape(NTOK, D)
    xt = np.ascontiguousarray(x.T).astype(ml_dtypes.bfloat16)  # [D, NTOK]
    r = xt.reshape(NCH, 128, NTOK)
    xt1 = np.ascontiguousarray(r[:, :, 0:QH].transpose(1, 0, 2).reshape(128, NCH * QH))
    xt2 = np.ascontiguousarray(r[:, :, QH:S].transpose(1, 0, 2).reshape(128, NCH * QH))
    xt3 = np.ascontiguousarray(r[:, :, S : 2 * S].transpose(1, 0, 2).reshape(128, NCH * S))
    in_maps = []
    for c in range(NCORES):
        lo, hi = c * FPC, (c + 1) * FPC
        wall = np.zeros((128, WALLW), dtype=np.float32)
        for ch in range(NCH):
            rs = slice(ch * 128, (ch + 1) * 128)
            wall[:, ch * FPC : (ch + 1) * FPC] = WK[rs, lo:hi]
            wall[:, NCH * FPC + ch * FPC : NCH * FPC + (ch + 1) * FPC] = WQ[rs, lo:hi]
            o = 2 * NCH * FPC
            wall[:, o + ch * VW : o + ch * VW + HS] = WV[rs, lo : lo + HS]
            wall[:, o + ch * VW + HS + 1 : o + ch * VW + 2 * HS + 1] = WV[rs, lo + HS : hi]
        bcol = np.stack([bQ[lo:hi], bK[lo:hi]], axis=1).astype(np.float32)
        brow = np.zeros((1, VW + KT), dtype=np.float32)
        brow[0, 0:HS] = bV[lo : lo + HS]
        brow[0, HS] = 1.0
        brow[0, HS + 1 : 2 * HS + 1] = bV[lo + HS : hi]
        brow[0, 2 * HS + 1] = 1.0
        brow[0, VW:] = 1.0
        in_maps.append(
            {
                "xt1": xt1,
                "xt2": xt2,
                "xt3": xt3,
                "wall": wall.astype(ml_dtypes.bfloat16),
                "bcol": np.ascontiguousarray(bcol),
                "brow": brow.astype(ml_dtypes.bfloat16),
            }
        )
    return in_maps


def run(in_maps, trace=False):
    nc = get_nc()
    return bass_utils.run_bass_kernel_spmd(nc, in_maps, core_ids=list(range(NCORES)), trace=trace)


def kernel(seq_input, WQ, bQ, WK, bK, WV, bV):
    in_maps = make_in_maps(
        np.asarray(seq_input, np.float32),
        np.asarray(WQ, np.float32), np.asarray(bQ, np.float32),
        np.asarray(WK, np.float32), np.asarray(bK, np.float32),
        np.asarray(WV, np.float32), np.asarray(bV, np.float32),
    )
    res = run(in_maps)
    parts = []
    for c in range(NCORES):
        o = res.results[c]["out"]  # [130, 4096] feature-major, unnormalized
        for h in range(2):
            num = o[h * (HS + 1) : h * (HS + 1) + HS, :]      # [64, 4096]
            den = o[h * (HS + 1) + HS, :]                     # [4096]
            parts.append((num / den).T)                       # [4096, 64]
    full = np.concatenate(parts, axis=1)  # [4096, 1024]
    return full.reshape(B, S, H * HS)
